# revision 8
# baseline (speedup 1.0000x reference)
"""DST-II kernel for Trainium2 (8 NeuronCores, Bass/Tile).

y[m, k] = sum_n x[m, n] * sin(pi/N * (n + 1/2) * (k + 1)),  x: [16384, 1024] f32.

This is a batched matmul y = x @ S with a fixed [1024, 1024] sine table.
Sharding: batch (rows of x) split across 8 cores, S replicated.

Fast-DST folding: S has the row symmetry S[N-1-n, k] = (-1)^k S[n, k], so
with u = x_front + x_back_rev, v = x_front - x_back_rev:
    y[:, 0::2] = u @ A,  A = S[:512, 0::2]          (512x512)
    y[:, 1::2] = v @ B,  B = S[:512, 1::2]          (512x512)
B is itself a DST-II-style kernel with the same symmetry, so the v branch
folds once more (p = fold+(v), q = fold-(v)):
    y[:, 1::4] = p @ B[:256, 0::2],   y[:, 3::4] = q @ B[:256, 1::2]
This removes 3/8 of the matmul FLOPs and 5/8 of the table traffic. Folds
run on-device (DVE for level 1, GpSimd for level 2). A row permutation pi
of the folded space keeps level-2 fold partners partition-aligned; the
table rows are permuted identically on the host (contraction is
order-invariant).

Implementation notes:
  - TensorE computes out = lhsT.T @ rhs with the contraction dim on
    partitions, so the stationary operand must be x^T-layout; each core's
    row-slab is shipped pre-transposed/pre-permuted, packed so every chunk
    DMA is one contiguous run per partition.
  - Matmuls run in float32r (TF32-like, 2 cycles/row for 4-byte operands,
    ~1.8e-4 rel err). Inputs are declared float32r in DRAM directly; the
    hardware accepts raw fp32 bits with accuracy identical to pre-rounded
    data.
  - Chunk sizes ramp 128..512..128 to shorten the serial head/tail.
  - PSUM->SBUF copies interleave the output column classes via strided
    APs, split across Scalar and Vector engines.
"""

import numpy as np
from contextlib import ExitStack

import concourse.bass as bass
import concourse.mybir as mybir
import concourse.tile as tile
from concourse import bacc
from concourse.bass_utils import run_bass_kernel_spmd

N_CORES = 8
B = 16384            # total batch (rows)
N = 1024             # transform length
M_CORE = B // N_CORES   # rows per core = 2048
P = 128
NH = N // 2          # level-1 folded length = 512
NQ = N // 4          # level-2 folded length = 256
CHUNKS = [128, 128, 256, 512, 512, 256, 128, 128]
MAX_CHUNK = max(CHUNKS)
assert sum(CHUNKS) == M_CORE

# permutation of the folded space: tiles [0:128], [128:256], [383:255:-1],
# [511:383:-1] — aligns level-2 fold partners (n', 511-n') across tiles.
PI = np.concatenate([np.arange(0, 128), np.arange(128, 256),
                     np.arange(383, 255, -1), np.arange(511, 383, -1)])

_CACHE = {}


def _dst_table() -> np.ndarray:
    n = np.arange(N, dtype=np.float64)
    k = np.arange(N, dtype=np.float64)
    return np.sin((np.pi / N) * (n[:, None] + 0.5) * (k[None, :] + 1.0))


def _tables():
    S = _dst_table()
    A = np.ascontiguousarray(S[:NH, 0::2][PI, :].astype(np.float32))
    Bm = S[:NH, 1::2]
    BP = np.ascontiguousarray(Bm[:NQ, 0::2].astype(np.float32))
    BQ = np.ascontiguousarray(Bm[:NQ, 1::2].astype(np.float32))
    return A, BP, BQ


def _build():
    f32 = mybir.dt.float32
    f32r = mybir.dt.float32r
    nc = bacc.Bacc("TRN2", target_bir_lowering=False, debug=False,
                   enable_asserts=False)
    xP = nc.dram_tensor("xP", [P, 8 * M_CORE], f32r, kind="ExternalInput").ap()
    A = nc.dram_tensor("A", [NH, NH], f32r, kind="ExternalInput").ap()
    BP = nc.dram_tensor("BP", [NQ, NQ], f32r, kind="ExternalInput").ap()
    BQ = nc.dram_tensor("BQ", [NQ, NQ], f32r, kind="ExternalInput").ap()
    y = nc.dram_tensor("y", [M_CORE, N], f32, kind="ExternalOutput").ap()

    with tile.TileContext(nc) as tc:
        with ExitStack() as ctx:
            const = ctx.enter_context(tc.tile_pool(name="const", bufs=1))
            xin = ctx.enter_context(tc.tile_pool(name="xin", bufs=4))
            fold = ctx.enter_context(tc.tile_pool(name="fold", bufs=2))
            yout = ctx.enter_context(tc.tile_pool(name="yout", bufs=2))
            ps = ctx.enter_context(tc.tile_pool(name="ps", bufs=2, space="PSUM"))

            # A tiles first (the first psum group needs all of A); BP/BQ are
            # issued after the first x chunk, hidden under the u-branch MMs.
            A_t = []
            for k in range(4):
                t = const.tile([P, NH], f32r, tag=f"A{k}")
                nc.sync.dma_start(t[:], A[k * P:(k + 1) * P, :])
                A_t.append(t)

            BP_t, BQ_t = [], []

            def load_b():
                for k in range(2):
                    t = const.tile([P, NQ], f32r, tag=f"BP{k}")
                    nc.sync.dma_start(t[:], BP[k * P:(k + 1) * P, :])
                    BP_t.append(t)
                for k in range(2):
                    t = const.tile([P, NQ], f32r, tag=f"BQ{k}")
                    nc.sync.dma_start(t[:], BQ[k * P:(k + 1) * P, :])
                    BQ_t.append(t)

            m0 = 0
            for ci, mc in enumerate(CHUNKS):
                w = mc  # per-tile column width of this chunk
                xc = xin.tile([P, 8 * MAX_CHUNK], f32r, tag="xc")
                nc.sync.dma_start(xc[:, :8 * w], xP[:, 8 * m0:8 * (m0 + w)])
                if ci == 0:
                    load_b()
                u = fold.tile([P, 4 * MAX_CHUNK], f32r, tag="u")
                v = fold.tile([P, 4 * MAX_CHUNK], f32r, tag="v")
                nc.vector.tensor_add(u[:, :4 * w], xc[:, :4 * w],
                                     xc[:, 4 * w:8 * w])
                nc.vector.tensor_sub(v[:, :4 * w], xc[:, :4 * w],
                                     xc[:, 4 * w:8 * w])
                p = fold.tile([P, 2 * MAX_CHUNK], f32r, tag="p")
                q = fold.tile([P, 2 * MAX_CHUNK], f32r, tag="q")
                nc.gpsimd.tensor_add(p[:, :w], v[:, :w], v[:, 3 * w:4 * w])
                nc.gpsimd.tensor_add(p[:, w:2 * w], v[:, w:2 * w],
                                     v[:, 2 * w:3 * w])
                nc.gpsimd.tensor_sub(q[:, :w], v[:, :w], v[:, 3 * w:4 * w])
                nc.gpsimd.tensor_sub(q[:, w:2 * w], v[:, w:2 * w],
                                     v[:, 2 * w:3 * w])

                yc = yout.tile([P, MAX_CHUNK // P, N], f32, tag="yc")
                for mt in range(mc // P):
                    ym2 = yc[:, mt, :].rearrange("p (i two) -> p i two", two=2)
                    ym4 = yc[:, mt, :].rearrange("p (i four) -> p i four",
                                                 four=4)
                    acc_e = ps.tile([P, NH], f32, tag="acc_e")
                    for k in range(4):
                        nc.tensor.matmul(
                            acc_e[:], u[:, k * w + mt * P:k * w + mt * P + P],
                            A_t[k][:], start=(k == 0), stop=(k == 3))
                    nc.scalar.copy(out=ym2[:, :, 0], in_=acc_e[:])
                    acc_p = ps.tile([P, NQ], f32, tag="acc_p")
                    for k in range(2):
                        nc.tensor.matmul(
                            acc_p[:], p[:, k * w + mt * P:k * w + mt * P + P],
                            BP_t[k][:], start=(k == 0), stop=(k == 1))
                    nc.scalar.copy(out=ym4[:, :, 1], in_=acc_p[:])
                    acc_q = ps.tile([P, NQ], f32, tag="acc_q")
                    for k in range(2):
                        nc.tensor.matmul(
                            acc_q[:], q[:, k * w + mt * P:k * w + mt * P + P],
                            BQ_t[k][:], start=(k == 0), stop=(k == 1))
                    nc.vector.tensor_copy(out=ym4[:, :, 3], in_=acc_q[:])
                nc.sync.dma_start(
                    y[m0:m0 + mc, :].rearrange("(o p) f -> p o f", p=P),
                    yc[:, :mc // P, :])
                m0 += mc

    nc.compile()
    return nc


def _get_nc():
    if "nc" not in _CACHE:
        _CACHE["nc"] = _build()
    return _CACHE["nc"]


def _pack_x(xs: np.ndarray) -> np.ndarray:
    """[M_CORE, N] row-slab -> packed [128, 8*M_CORE] fold-ready layout."""
    front = xs[:, PI].T                  # [512, m]
    back = xs[:, 1023 - PI].T            # [512, m]
    xT2 = np.concatenate([front, back], axis=0)   # [1024, m]
    blocks = []
    m0 = 0
    for mc in CHUNKS:
        blk = xT2[:, m0:m0 + mc].reshape(8, P, mc)
        blocks.append(blk.transpose(1, 0, 2).reshape(P, 8 * mc))
        m0 += mc
    return np.ascontiguousarray(np.concatenate(blocks, axis=1))


def _in_maps(x: np.ndarray):
    if "tabs" not in _CACHE:
        _CACHE["tabs"] = _tables()
    A, BP, BQ = _CACHE["tabs"]
    x = np.ascontiguousarray(x, dtype=np.float32)
    maps = []
    for c in range(N_CORES):
        xs = x[c * M_CORE:(c + 1) * M_CORE]
        maps.append({"xP": _pack_x(xs), "A": A, "BP": BP, "BQ": BQ})
    return maps


def kernel(x: np.ndarray) -> np.ndarray:
    nc = _get_nc()
    res = run_bass_kernel_spmd(nc, _in_maps(x), list(range(N_CORES)))
    return np.concatenate([res.results[c]["y"] for c in range(N_CORES)], axis=0)


def _install_profile_hooks():
    """The agent image's antenv lacks axon_hooks; recreate it from
    trn_agent_boot so run_bass_kernel_spmd(trace=True) can capture NTFF
    profiles. Also stub out the S3 artifact upload."""
    import sys, types
    import concourse.bass_utils as bu

    if "antenv.axon_hooks" not in sys.modules:
        from trn_agent_boot.trn_boot import _ntff_profile_via_ctypes
        hook = _ntff_profile_via_ctypes("/opt/axon/libaxon_pjrt.so")
        mod = types.ModuleType("antenv.axon_hooks")
        mod.get_axon_ntff_profile_hook = lambda: hook
        mod.set_axon_ntff_profile_hook = lambda h: None
        sys.modules["antenv.axon_hooks"] = mod
    bu.upload_artifacts = lambda tmpdir: f"local:{tmpdir}"


def profile(x: np.ndarray, tmpdir=None, trace_kwargs={}):
    """Run once with NTFF tracing; returns (exec_time_ns, BassKernelResults)."""
    _install_profile_hooks()
    nc = _get_nc()
    res = run_bass_kernel_spmd(nc, _in_maps(x), list(range(N_CORES)),
                               trace=True, tmpdir=tmpdir,
                               trace_kwargs=trace_kwargs)
    return res.exec_time_ns, res


# revision 9
# speedup vs baseline: 1.1812x; 1.1812x over previous
"""DST-II kernel for Trainium2 (8 NeuronCores, Bass/Tile).

y[m, k] = sum_n x[m, n] * sin(pi/N * (n + 1/2) * (k + 1)),  x: [16384, 1024] f32.

This is a batched matmul y = x @ S with a fixed [1024, 1024] sine table.
Sharding: batch (rows of x) split across 8 cores, S replicated.

Fold optimization: S has the row symmetry S[N-1-n, k] = (-1)^k S[n, k], so
with u = x_front + x_back_rev, v = x_front - x_back_rev:
    y[:, 0::2] = u @ A,  A = S[:512, 0::2]          (512x512)
    y[:, 1::2] = v @ B,  B = S[:512, 1::2]          (512x512)
which halves the matmul FLOPs and the table traffic. Folds run on the
vector engine in fp32.

Implementation notes:
  - TensorE computes out = lhsT.T @ rhs with the contraction dim on
    partitions, so the stationary operand must be x^T-layout; each core's
    row-slab is shipped pre-transposed (second half of columns reversed),
    packed so every chunk DMA is one contiguous run per partition.
  - Matmuls run in float32r (TF32-like, ~2 cycles/row for 4-byte operands,
    ~1.8e-4 rel err). Inputs are declared float32r in DRAM directly; the
    hardware accepts raw fp32 bits with accuracy identical to pre-rounded
    data.
  - Chunk sizes ramp 128..512..128 to shorten the serial head/tail.
  - x loads + table loads issue on the Sync HWDGE queue; y stores issue on
    the Scalar HWDGE queue so a store waiting on compute never head-of-line
    blocks the next chunk's load.
  - PSUM->SBUF copies interleave even/odd output columns via stride-2 APs,
    on the Scalar engine (ACT) to keep the DVE free for folds.
"""

import numpy as np
from contextlib import ExitStack

import concourse.bass as bass
import concourse.mybir as mybir
import concourse.tile as tile
from concourse import bacc
from concourse.bass_utils import run_bass_kernel_spmd

N_CORES = 8
B = 16384            # total batch (rows)
N = 1024             # transform length
M_CORE = B // N_CORES   # rows per core = 2048
P = 128
NH = N // 2          # folded contraction length = 512
CHUNKS = [128, 128, 256, 512, 512, 256, 128, 128]
MAX_CHUNK = max(CHUNKS)
assert sum(CHUNKS) == M_CORE

_CACHE = {}


def _dst_table() -> np.ndarray:
    n = np.arange(N, dtype=np.float64)
    k = np.arange(N, dtype=np.float64)
    return np.sin((np.pi / N) * (n[:, None] + 0.5) * (k[None, :] + 1.0))


def _tables():
    S = _dst_table()
    A = np.ascontiguousarray(S[:NH, 0::2].astype(np.float32))
    Bm = np.ascontiguousarray(S[:NH, 1::2].astype(np.float32))
    return A, Bm


def _build():
    f32 = mybir.dt.float32
    f32r = mybir.dt.float32r
    nc = bacc.Bacc("TRN2", target_bir_lowering=False, debug=False,
                   enable_asserts=False)
    xP = nc.dram_tensor("xP", [P, 8 * M_CORE], f32r, kind="ExternalInput").ap()
    A = nc.dram_tensor("A", [NH, NH], f32r, kind="ExternalInput").ap()
    Bm = nc.dram_tensor("Bm", [NH, NH], f32r, kind="ExternalInput").ap()
    y = nc.dram_tensor("y", [M_CORE, N], f32, kind="ExternalOutput").ap()

    with tile.TileContext(nc) as tc:
        with ExitStack() as ctx:
            const = ctx.enter_context(tc.tile_pool(name="const", bufs=1))
            xin = ctx.enter_context(tc.tile_pool(name="xin", bufs=4))
            fold = ctx.enter_context(tc.tile_pool(name="fold", bufs=2))
            yout = ctx.enter_context(tc.tile_pool(name="yout", bufs=2))
            ps = ctx.enter_context(tc.tile_pool(name="ps", bufs=6, space="PSUM"))

            # A tiles first (the first psum group needs all of A); B tiles
            # are issued after the first x chunk, hidden under u-branch MMs.
            A_t, B_t = [], []
            for k in range(4):
                t = const.tile([P, NH], f32r, tag=f"A{k}")
                nc.sync.dma_start(t[:], A[k * P:(k + 1) * P, :])
                A_t.append(t)

            m0 = 0
            for ci, mc in enumerate(CHUNKS):
                w = mc
                xc = xin.tile([P, 8 * MAX_CHUNK], f32r, tag="xc")
                nc.sync.dma_start(xc[:, :8 * w], xP[:, 8 * m0:8 * (m0 + w)])
                if ci == 0:
                    for k in range(4):
                        t = const.tile([P, NH], f32r, tag=f"B{k}")
                        nc.sync.dma_start(t[:], Bm[k * P:(k + 1) * P, :])
                        B_t.append(t)
                u = fold.tile([P, 4 * MAX_CHUNK], f32r, tag="u")
                v = fold.tile([P, 4 * MAX_CHUNK], f32r, tag="v")
                nc.vector.tensor_add(u[:, :4 * w], xc[:, :4 * w],
                                     xc[:, 4 * w:8 * w])
                nc.vector.tensor_sub(v[:, :4 * w], xc[:, :4 * w],
                                     xc[:, 4 * w:8 * w])

                yc = yout.tile([P, MAX_CHUNK // P, N], f32, tag="yc")
                for mt in range(mc // P):
                    ym = yc[:, mt, :].rearrange("p (i two) -> p i two", two=2)
                    for t, src, tab in ((0, u, A_t), (1, v, B_t)):
                        acc = ps.tile([P, NH], f32, tag="acc")
                        for k in range(4):
                            nc.tensor.matmul(
                                acc[:],
                                src[:, k * w + mt * P:k * w + mt * P + P],
                                tab[k][:], start=(k == 0), stop=(k == 3))
                        nc.scalar.copy(out=ym[:, :, t], in_=acc[:])
                nc.scalar.dma_start(
                    y[m0:m0 + mc, :].rearrange("(o p) f -> p o f", p=P),
                    yc[:, :mc // P, :])
                m0 += mc

    nc.compile()
    return nc


def _get_nc():
    if "nc" not in _CACHE:
        _CACHE["nc"] = _build()
    return _CACHE["nc"]


def _pack_x(xs: np.ndarray) -> np.ndarray:
    """[M_CORE, N] row-slab -> packed [128, 8*M_CORE] fold-ready layout."""
    front = xs[:, :NH].T                 # [512, m]
    back = xs[:, :NH - 1:-1].T           # [512, m] (columns reversed)
    xT2 = np.concatenate([front, back], axis=0)   # [1024, m]
    blocks = []
    m0 = 0
    for mc in CHUNKS:
        blk = xT2[:, m0:m0 + mc].reshape(8, P, mc)
        blocks.append(blk.transpose(1, 0, 2).reshape(P, 8 * mc))
        m0 += mc
    return np.ascontiguousarray(np.concatenate(blocks, axis=1))


def _in_maps(x: np.ndarray):
    if "tabs" not in _CACHE:
        _CACHE["tabs"] = _tables()
    A, Bm = _CACHE["tabs"]
    x = np.ascontiguousarray(x, dtype=np.float32)
    maps = []
    for c in range(N_CORES):
        xs = x[c * M_CORE:(c + 1) * M_CORE]
        maps.append({"xP": _pack_x(xs), "A": A, "Bm": Bm})
    return maps


def kernel(x: np.ndarray) -> np.ndarray:
    nc = _get_nc()
    res = run_bass_kernel_spmd(nc, _in_maps(x), list(range(N_CORES)))
    return np.concatenate([res.results[c]["y"] for c in range(N_CORES)], axis=0)


def _install_profile_hooks():
    """The agent image's antenv lacks axon_hooks; recreate it from
    trn_agent_boot so run_bass_kernel_spmd(trace=True) can capture NTFF
    profiles. Also stub out the S3 artifact upload."""
    import sys, types
    import concourse.bass_utils as bu

    if "antenv.axon_hooks" not in sys.modules:
        from trn_agent_boot.trn_boot import _ntff_profile_via_ctypes
        hook = _ntff_profile_via_ctypes("/opt/axon/libaxon_pjrt.so")
        mod = types.ModuleType("antenv.axon_hooks")
        mod.get_axon_ntff_profile_hook = lambda: hook
        mod.set_axon_ntff_profile_hook = lambda h: None
        sys.modules["antenv.axon_hooks"] = mod
    bu.upload_artifacts = lambda tmpdir: f"local:{tmpdir}"


def profile(x: np.ndarray, tmpdir=None, trace_kwargs={}):
    """Run once with NTFF tracing; returns (exec_time_ns, BassKernelResults)."""
    _install_profile_hooks()
    nc = _get_nc()
    res = run_bass_kernel_spmd(nc, _in_maps(x), list(range(N_CORES)),
                               trace=True, tmpdir=tmpdir,
                               trace_kwargs=trace_kwargs)
    return res.exec_time_ns, res
